# revision 12
# baseline (speedup 1.0000x reference)
"""Trainium2 Bass kernel for CheemsNonWoAttention (GQA attention, no output proj).

Sharding (v3): one (kv-head, batch) pair per core — 4 kv heads x 2 batches =
8 cores.  Each core projects K/V for its kv head and Q for the 4 q-heads of
that GQA group, over ONE batch only.  No work is duplicated anywhere (the
per-core PE load sits at the 8-way MAC roofline) and no collectives are
needed; the host concatenates per-core output slices.

Kernel design:
  - All matmuls bf16 (same 1 cycle/row PE rate as fp32r at moving>=256, but
    full rate at ANY even moving width; DMA and SBUF traffic halved).
  - Scores computed transposed, sT[k, q], per 512-q block; per k-chunk the
    moving range starts at max(q0, kc*128) (causal), so the upper-left
    rectangle of each diagonal block is never computed.  The remaining
    128x128 triangle is zeroed by a 0/1 multiply on the exp output.
  - exp on the scalar engine over 2-bank PSUM groups (FD=1024) to amortize
    the ~293ns per-ACTIVATE overhead.
  - Softmax denominators: partial sums over k accumulated on the vector
    engine in bf16 (2x mode) into two per-head panels (k-chunks 0-7 and
    8-15, capping accumulation depth at 8 for precision), DMA'd out raw;
    the HOST does the final 128-way partition reduction and the divide.
    No sums matmuls and no output transposes on the PE at all.
  - attn@V output staged PSUM->SBUF as bf16 (copies alternate between the
    scalar and vector engines) and DMA'd as oT[d, q]; host transposes and
    normalizes.
  - Causal pipelining: attention on q-block t0 only needs K/V for tokens
    <= t0+512, so each projection tile is immediately followed by its
    attention block and the engines overlap across the whole kernel.
"""

import sys

if "/opt/trn_rl_repo" not in sys.path:
    sys.path.insert(0, "/opt/trn_rl_repo")

import math
import numpy as np

B, S, HID = 2, 2048, 2048
NH, NKV, HD = 16, 4, 128
NCORES = 8
HPC = 4                     # q heads per core (one GQA group)
FPC = HPC * HD              # 512 output features per core
KVW = HD                    # kv head width per core
P = 128
NCH = HID // P              # hid chunks (contraction tiles)
TT = 512                    # token tile, phase 1
QT = 512                    # q block, phase 2
NKC = S // P                # k chunks
GK = 2                      # k-chunks per exp group (2 PSUM banks)
XSUB = 4                    # hid chunks per x sub-tile
NSUB = NCH // XSUB

_CACHE = {}


def _patch_ldw_opt():
    # walrus's LDWEIGHTS dedup/overlap pass is off by default in the driver
    # args; weight loads otherwise throttle back-to-back matmul issue.
    import concourse.bass_utils as bu

    if getattr(bu, "_ldw_opt_patched", False):
        return
    orig = bu.run_command

    def patched(argv, **kw):
        argv = ["--enable-ldw-opt=true" if a == "--enable-ldw-opt=false" else a
                for a in argv]
        return orig(argv, **kw)

    bu.run_command = patched
    bu._ldw_opt_patched = True


def _build_nc(variant):
    # NOTE: ldw-opt is left OFF — walrus rejects it for bf16 LDWEIGHTS
    # ("InstLdweights is not compatible with LDW optimization"); bf16 weight
    # loads get the compiler-automatic FWL fast path instead.
    import concourse.bacc as bacc
    from concourse import mybir
    from concourse.tile import TileContext

    f32 = mybir.dt.float32
    f32r = mybir.dt.float32r
    bf16 = mybir.dt.bfloat16
    Exp = mybir.ActivationFunctionType.Exp

    nc = bacc.Bacc("TRN2", target_bir_lowering=False, debug=False, num_devices=NCORES)
    # x pre-tiled on host to [tile, P, NCH, TT]; weights pre-arranged to
    # the exact SBUF layout [P, NCH, F] — DMAs become 128 fat contiguous
    # segments instead of thousands of 1KB rows (SWDGE descriptor cost).
    xt_d = nc.dram_tensor("xt", [S // TT, P, NCH, TT], bf16, kind="ExternalInput").ap()
    wq = nc.dram_tensor("wq", [P, NCH, FPC], bf16, kind="ExternalInput").ap()
    wk = nc.dram_tensor("wk", [P, NCH, KVW], bf16, kind="ExternalInput").ap()
    wv = nc.dram_tensor("wv", [P, NCH, KVW], bf16, kind="ExternalInput").ap()
    identb_d = nc.dram_tensor("identb", [P, P], f32r, kind="ExternalInput").ap()
    tri_d = nc.dram_tensor("tri", [P, P], bf16, kind="ExternalInput").ap()
    if variant == "general":
        maskT = nc.dram_tensor("maskT", [S, S], bf16, kind="ExternalInput").ap()
    oT_d = nc.dram_tensor("oT", [FPC, S], bf16, kind="ExternalOutput").ap()
    accL_d = nc.dram_tensor("accL", [HPC, P, S], bf16, kind="ExternalOutput").ap()
    accH_d = nc.dram_tensor("accH", [HPC, P, S], bf16, kind="ExternalOutput").ap()

    def n_kchunks(q0):
        if variant == "causal":
            return q0 // P + QT // P
        return NKC

    def q_start(q0, kc):
        # first valid (unmasked) q column for this k-chunk
        if variant == "causal":
            return max(q0, kc * P)
        return q0

    with TileContext(nc) as tc:
        with tc.tile_pool(name="persist", bufs=1) as persist, \
             tc.tile_pool(name="xt", bufs=2) as xpool, \
             tc.tile_pool(name="vst", bufs=2) as vstage, \
             tc.tile_pool(name="ost", bufs=3) as ostage, \
             tc.tile_pool(name="et", bufs=3) as etpool, \
             tc.tile_pool(name="acc", bufs=2 * HPC) as accpool, \
             tc.tile_pool(name="mask", bufs=4) as mpool, \
             tc.tile_pool(name="ppsum", bufs=2, space="PSUM") as ppsum, \
             tc.tile_pool(name="spsum", bufs=2, space="PSUM") as spsum, \
             tc.tile_pool(name="popsum", bufs=2, space="PSUM") as popool:
            wq_sb = persist.tile([P, NCH, FPC], bf16, tag="wq")
            wk_sb = persist.tile([P, NCH, KVW], bf16, tag="wk")
            wv_sb = persist.tile([P, NCH, KVW], bf16, tag="wv")
            identb = persist.tile([P, P], f32r, tag="identb")
            tri_sb = persist.tile([P, P], bf16, tag="tri")
            qT_sb = persist.tile([P, HPC, S], bf16, tag="qT")
            kT_sb = persist.tile([P, S], bf16, tag="kT")
            v_sb = persist.tile([P, S], bf16, tag="v")

            # Startup choreography: x streams on the sync/gpsimd queues
            # (emitted first by proj_tile(0)); weights ride the otherwise-idle
            # scalar queue in chunk-group pieces so the first Q matmul only
            # waits on ~1MB of input.
            for cg in range(0, NCH, XSUB):
                nc.scalar.dma_start(out=wq_sb[:, cg:cg + XSUB, :],
                                    in_=wq[:, cg:cg + XSUB, :])
            nc.scalar.dma_start(out=wk_sb[:], in_=wk[:])
            nc.scalar.dma_start(out=wv_sb[:], in_=wv[:])
            nc.scalar.dma_start(out=identb[:], in_=identb_d[:])
            nc.scalar.dma_start(out=tri_sb[:], in_=tri_d[:])

            def proj_tile(t0):
                ti = t0 // TT
                xs = xpool.tile([P, NCH, TT], bf16, tag="xt", name=f"xt_{t0}")
                nc.sync.dma_start(out=xs[:, :NCH // 2, :],
                                  in_=xt_d[ti, :, :NCH // 2, :])
                nc.gpsimd.dma_start(out=xs[:, NCH // 2:, :],
                                    in_=xt_d[ti, :, NCH // 2:, :])
                for h in range(HPC):
                    ps = ppsum.tile([P, TT], f32, tag="pp")
                    for c in range(NCH):
                        nc.tensor.matmul(
                            ps[:],
                            lhsT=wq_sb[:, c, h * HD:(h + 1) * HD],
                            rhs=xs[:, c, :],
                            start=(c == 0), stop=(c == NCH - 1),
                        )
                    nc.vector.tensor_copy(qT_sb[:, h, t0:t0 + TT], ps[:])
                ps = ppsum.tile([P, TT], f32, tag="pp")
                for c in range(NCH):
                    nc.tensor.matmul(
                        ps[:], lhsT=wk_sb[:, c, :], rhs=xs[:, c, :],
                        start=(c == 0), stop=(c == NCH - 1),
                    )
                nc.vector.tensor_copy(kT_sb[:, t0:t0 + TT], ps[:])
                ps = ppsum.tile([P, TT], f32, tag="pp")
                for c in range(NCH):
                    nc.tensor.matmul(
                        ps[:], lhsT=wv_sb[:, c, :], rhs=xs[:, c, :],
                        start=(c == 0), stop=(c == NCH - 1),
                    )
                vt = vstage.tile([P, TT], f32r, tag="vt", name=f"vt_{t0}")
                nc.scalar.copy(vt[:], ps[:])
                vtp = popool.tile([P, QT], f32r, tag="po", name=f"vtp_{t0}")
                for j in range(TT // P):
                    nc.tensor.transpose(
                        vtp[:, j * P:(j + 1) * P], vt[:, j * P:(j + 1) * P], identb[:]
                    )
                nc.vector.tensor_copy(v_sb[:, t0:t0 + TT], vtp[:])

            acc_tiles = {}

            def attn_block(q0):
                # Software-pipelined emission: score-groups of head h are
                # interleaved with attn@V chunk-pairs (and DVE sum-adds) of
                # head h-1, so the PE never sits out a full exp latency and
                # the DVE work is spread evenly.
                nkc = n_kchunks(q0)
                ngr = (nkc + GK - 1) // GK
                ets, pos = {}, {}

                def scores_group(h, g0):
                    sp = spsum.tile([P, GK * QT], f32, tag="sp")
                    for kc in range(g0, min(g0 + GK, nkc)):
                        qs = q_start(q0, kc)
                        off = (kc - g0) * QT + (qs - q0)
                        nc.tensor.matmul(
                            sp[:, off:(kc - g0) * QT + QT],
                            lhsT=kT_sb[:, kc * P:(kc + 1) * P],
                            rhs=qT_sb[:, h, qs:q0 + QT],
                            start=True, stop=True,
                        )
                    if variant == "general":
                        mt = mpool.tile([P, GK, QT], bf16, tag="mt")
                        nc.sync.dma_start(
                            out=mt[:],
                            in_=maskT[g0 * P:(g0 + GK) * P, q0:q0 + QT]
                            .rearrange("(g p) q -> p g q", p=P),
                        )
                        nc.vector.tensor_add(out=sp[:], in0=sp[:], in1=mt[:])
                    nc.scalar.activation(
                        out=ets[h][:, g0 * QT:(g0 + GK) * QT], in_=sp[:], func=Exp
                    )
                    if variant == "causal":
                        for kc in range(g0, min(g0 + GK, nkc)):
                            if kc * P >= q0:
                                off = kc * QT + (kc * P - q0)
                                nc.vector.tensor_mul(
                                    out=ets[h][:, off:off + P],
                                    in0=ets[h][:, off:off + P],
                                    in1=tri_sb[:],
                                )

                def attnv_part(h, g0):
                    if h not in pos:
                        pos[h] = popool.tile([P, QT], f32, tag="po",
                                             name=f"po_{q0}_{h}")
                    for kc in range(g0, min(g0 + GK, nkc)):
                        qs = q_start(q0, kc)
                        off = qs - q0
                        nc.tensor.matmul(
                            pos[h][:, off:QT],
                            lhsT=v_sb[:, kc * P:(kc + 1) * P],
                            rhs=ets[h][:, kc * QT + off:kc * QT + QT],
                            start=(kc == 0), stop=(kc == nkc - 1),
                        )

                def finish_head(h):
                    # ot copy rides the scalar engine and is emitted BEFORE
                    # the DVE sum-adds: the po PSUM rotation (which gates the
                    # PE) must never wait behind queued DVE work.
                    accL, accH = acc_tiles[h]
                    ot = ostage.tile([P, QT], bf16, tag="ot", name=f"ot_{q0}_{h}")
                    nc.scalar.copy(ot[:], pos[h][:])
                    nc.gpsimd.dma_start(
                        out=oT_d[h * P:(h + 1) * P, q0:q0 + QT], in_=ot[:]
                    )
                    # softmax partial sums on DVE (bf16, depth <= 8)
                    for kc in range(nkc):
                        qs = q_start(q0, kc)
                        off = qs - q0
                        acc = accL if kc < 8 else accH
                        et_s = ets[h][:, kc * QT + off:kc * QT + QT]
                        dst = acc[:, qs:q0 + QT]
                        if kc == 0 or kc == 8:
                            nc.vector.tensor_copy(dst, et_s)
                        else:
                            nc.vector.tensor_add(out=dst, in0=dst, in1=et_s)
                    if q0 == S - QT:
                        nc.gpsimd.dma_start(out=accL_d[h], in_=accL[:])
                        nc.gpsimd.dma_start(out=accH_d[h], in_=accH[:])

                for h in range(HPC):
                    if h not in acc_tiles:
                        acc_tiles[h] = (
                            accpool.tile([P, S], bf16, tag="acc", name=f"accL_{h}"),
                            accpool.tile([P, S], bf16, tag="acc", name=f"accH_{h}"),
                        )
                for h in range(HPC + 1):
                    if h < HPC:
                        ets[h] = etpool.tile([P, NKC * QT], bf16, tag="et",
                                             name=f"et_{q0}_{h}")
                    for g in range(ngr):
                        if h < HPC:
                            scores_group(h, g * GK)
                        if h >= 1:
                            attnv_part(h - 1, g * GK)
                    if h >= 1:
                        finish_head(h - 1)

            # causal pipeline: attention on block t0 needs only tokens <= t0+TT
            for t0 in range(0, S, TT):
                proj_tile(t0)
                attn_block(t0)

    nc.compile()
    return nc


def get_nc(variant="causal"):
    if variant not in _CACHE:
        _CACHE[variant] = _build_nc(variant)
    return _CACHE[variant]


def detect_variant(attention_mask):
    m = np.asarray(attention_mask, dtype=np.float32)[:, 0]   # [B, S, S] (q, k)
    if not np.any(m):
        return "zeros"
    kk = np.arange(S)
    lower = kk[None, :] <= kk[:, None]                       # [S(q), S(k)]
    for b in range(m.shape[0]):
        if np.any(m[b][lower] != 0.0):
            return "general"
        if np.any(m[b][~lower] > -1e8):
            return "general"
    return "causal"


def make_in_maps(hidden_states, attention_mask, Wq, Wk, Wv, variant=None):
    import ml_dtypes

    if variant is None:
        variant = detect_variant(attention_mask)
    bf = ml_dtypes.bfloat16
    x = np.asarray(hidden_states, dtype=np.float32)
    # [b] -> [S//TT, P, NCH, TT]: x[b].T is [HID, S]; hid -> (c p), s -> (ti t)
    xTb = [np.ascontiguousarray(
        x[b].T.reshape(NCH, P, S // TT, TT).transpose(2, 1, 0, 3)
    ).astype(bf) for b in range(B)]
    wq_s = (np.asarray(Wq, dtype=np.float32) / math.sqrt(HD)).astype(bf)
    wk = np.asarray(Wk, dtype=np.float32).astype(bf)
    wv = np.asarray(Wv, dtype=np.float32).astype(bf)

    def warr(w):  # [HID, F] -> [P, NCH, F]
        return np.ascontiguousarray(w.reshape(NCH, P, -1).transpose(1, 0, 2))
    identb = np.eye(P, dtype=np.float32)
    # tri[p, j] = 1 if p <= j else 0  (keep k <= q within the diagonal chunk)
    tri = np.triu(np.ones((P, P), dtype=np.float32)).astype(bf)
    if variant == "general":
        mT = [np.ascontiguousarray(
            np.asarray(attention_mask, dtype=np.float32)[b, 0].T).astype(bf)
            for b in range(B)]

    in_maps = []
    for c in range(NCORES):
        b, kv = c % 2, c // 2
        m = {
            "xt": xTb[b],
            "wq": warr(wq_s[:, kv * FPC:(kv + 1) * FPC]),
            "wk": warr(wk[:, kv * KVW:(kv + 1) * KVW]),
            "wv": warr(wv[:, kv * KVW:(kv + 1) * KVW]),
            "identb": identb,
            "tri": tri,
        }
        if variant == "general":
            m["maskT"] = mT[b]
        in_maps.append(m)
    return in_maps


def postprocess(res, variant):
    """Assemble full [B, S, HID] f32 output from per-core oT/accL/accH."""
    out = np.empty((B, S, HID), dtype=np.float32)
    for c in range(NCORES):
        b, kv = c % 2, c // 2
        oT = res.results[c]["oT"].astype(np.float64)          # [FPC, S]
        aL = res.results[c]["accL"].astype(np.float64)        # [HPC, P, S]
        aH = res.results[c]["accH"].astype(np.float64)
        if variant == "causal":
            # k-chunks 8..15 only reach q >= 1024; cols below hold garbage
            aH = aH.copy()
            aH[..., :8 * P] = 0.0
        sums = aL.sum(axis=1) + aH.sum(axis=1)                # [HPC, S]
        o = (oT.reshape(HPC, HD, S) / sums[:, None, :])       # [HPC, HD, S]
        out[b, :, kv * FPC:(kv + 1) * FPC] = (
            o.transpose(2, 0, 1).reshape(S, FPC).astype(np.float32)
        )
    return out


def run_on_cores(inputs, trace=False, tmpdir=None):
    from concourse.bass_utils import run_bass_kernel_spmd

    variant = detect_variant(inputs["attention_mask"])
    nc = get_nc(variant)
    in_maps = make_in_maps(**inputs, variant=variant)
    kw = {}
    if trace:
        kw = {"trace": True, "tmpdir": tmpdir}
    res = run_bass_kernel_spmd(nc, in_maps, core_ids=list(range(NCORES)), **kw)
    return postprocess(res, variant), res


def kernel(hidden_states, attention_mask, Wq, Wk, Wv):
    out, _ = run_on_cores({
        "hidden_states": hidden_states,
        "attention_mask": attention_mask,
        "Wq": Wq, "Wk": Wk, "Wv": Wv,
    })
    return out


# revision 13
# speedup vs baseline: 1.0155x; 1.0155x over previous
"""Trainium2 Bass kernel for CheemsNonWoAttention (GQA attention, no output proj).

Sharding (v3): one (kv-head, batch) pair per core — 4 kv heads x 2 batches =
8 cores.  Each core projects K/V for its kv head and Q for the 4 q-heads of
that GQA group, over ONE batch only.  No work is duplicated anywhere (the
per-core PE load sits at the 8-way MAC roofline) and no collectives are
needed; the host concatenates per-core output slices.

Kernel design:
  - All matmuls bf16 (same 1 cycle/row PE rate as fp32r at moving>=256, but
    full rate at ANY even moving width; DMA and SBUF traffic halved).
  - Scores computed transposed, sT[k, q], per 512-q block; per k-chunk the
    moving range starts at max(q0, kc*128) (causal), so the upper-left
    rectangle of each diagonal block is never computed.  The remaining
    128x128 triangle is zeroed by a 0/1 multiply on the exp output.
  - exp on the scalar engine over 2-bank PSUM groups (FD=1024) to amortize
    the ~293ns per-ACTIVATE overhead.
  - Softmax denominators: partial sums over k accumulated on the vector
    engine in bf16 (2x mode) into two per-head panels (k-chunks 0-7 and
    8-15, capping accumulation depth at 8 for precision), DMA'd out raw;
    the HOST does the final 128-way partition reduction and the divide.
    No sums matmuls and no output transposes on the PE at all.
  - attn@V output staged PSUM->SBUF as bf16 (copies alternate between the
    scalar and vector engines) and DMA'd as oT[d, q]; host transposes and
    normalizes.
  - Causal pipelining: attention on q-block t0 only needs K/V for tokens
    <= t0+512, so each projection tile is immediately followed by its
    attention block and the engines overlap across the whole kernel.
"""

import sys

if "/opt/trn_rl_repo" not in sys.path:
    sys.path.insert(0, "/opt/trn_rl_repo")

import math
import numpy as np

B, S, HID = 2, 2048, 2048
NH, NKV, HD = 16, 4, 128
NCORES = 8
HPC = 4                     # q heads per core (one GQA group)
FPC = HPC * HD              # 512 output features per core
KVW = HD                    # kv head width per core
P = 128
NCH = HID // P              # hid chunks (contraction tiles)
TT = 512                    # token tile, phase 1
QT = 512                    # q block, phase 2
NKC = S // P                # k chunks
GK = 2                      # k-chunks per exp group (2 PSUM banks)
XSUB = 4                    # hid chunks per x sub-tile
NSUB = NCH // XSUB

_CACHE = {}


def _patch_ldw_opt():
    # walrus's LDWEIGHTS dedup/overlap pass is off by default in the driver
    # args; weight loads otherwise throttle back-to-back matmul issue.
    import concourse.bass_utils as bu

    if getattr(bu, "_ldw_opt_patched", False):
        return
    orig = bu.run_command

    def patched(argv, **kw):
        argv = ["--enable-ldw-opt=true" if a == "--enable-ldw-opt=false" else a
                for a in argv]
        return orig(argv, **kw)

    bu.run_command = patched
    bu._ldw_opt_patched = True


def _build_nc(variant):
    # NOTE: ldw-opt is left OFF — walrus rejects it for bf16 LDWEIGHTS
    # ("InstLdweights is not compatible with LDW optimization"); bf16 weight
    # loads get the compiler-automatic FWL fast path instead.
    import concourse.bacc as bacc
    from concourse import mybir
    from concourse.tile import TileContext

    f32 = mybir.dt.float32
    f32r = mybir.dt.float32r
    bf16 = mybir.dt.bfloat16
    Exp = mybir.ActivationFunctionType.Exp

    nc = bacc.Bacc("TRN2", target_bir_lowering=False, debug=False, num_devices=NCORES)
    # x pre-tiled on host to [tile, P, NCH, TT]; weights pre-arranged to
    # the exact SBUF layout [P, NCH, F] — DMAs become 128 fat contiguous
    # segments instead of thousands of 1KB rows (SWDGE descriptor cost).
    xt_d = nc.dram_tensor("xt", [S // TT, P, NCH, TT], bf16, kind="ExternalInput").ap()
    wq = nc.dram_tensor("wq", [P, NCH, FPC], bf16, kind="ExternalInput").ap()
    wk = nc.dram_tensor("wk", [P, NCH, KVW], bf16, kind="ExternalInput").ap()
    wv = nc.dram_tensor("wv", [P, NCH, KVW], bf16, kind="ExternalInput").ap()
    identb_d = nc.dram_tensor("identb", [P, P], f32r, kind="ExternalInput").ap()
    tri_d = nc.dram_tensor("tri", [P, P], bf16, kind="ExternalInput").ap()
    if variant == "general":
        maskT = nc.dram_tensor("maskT", [S, S], bf16, kind="ExternalInput").ap()
    oT_d = nc.dram_tensor("oT", [FPC, S], bf16, kind="ExternalOutput").ap()
    accL_d = nc.dram_tensor("accL", [HPC, P, S], bf16, kind="ExternalOutput").ap()
    accH_d = nc.dram_tensor("accH", [HPC, P, S], bf16, kind="ExternalOutput").ap()

    def n_kchunks(q0):
        if variant == "causal":
            return q0 // P + QT // P
        return NKC

    def q_start(q0, kc):
        # first valid (unmasked) q column for this k-chunk
        if variant == "causal":
            return max(q0, kc * P)
        return q0

    with TileContext(nc) as tc:
        with tc.tile_pool(name="persist", bufs=1) as persist, \
             tc.tile_pool(name="xt", bufs=2) as xpool, \
             tc.tile_pool(name="vst", bufs=2) as vstage, \
             tc.tile_pool(name="ost", bufs=3) as ostage, \
             tc.tile_pool(name="et", bufs=3) as etpool, \
             tc.tile_pool(name="acc", bufs=2 * HPC) as accpool, \
             tc.tile_pool(name="mask", bufs=4) as mpool, \
             tc.tile_pool(name="ppsum", bufs=2, space="PSUM") as ppsum, \
             tc.tile_pool(name="spsum", bufs=2, space="PSUM") as spsum, \
             tc.tile_pool(name="popsum", bufs=2, space="PSUM") as popool:
            wq_sb = persist.tile([P, NCH, FPC], bf16, tag="wq")
            wk_sb = persist.tile([P, NCH, KVW], bf16, tag="wk")
            wv_sb = persist.tile([P, NCH, KVW], bf16, tag="wv")
            identb = persist.tile([P, P], f32r, tag="identb")
            tri_sb = persist.tile([P, P], bf16, tag="tri")
            qT_sb = persist.tile([P, HPC, S], bf16, tag="qT")
            kT_sb = persist.tile([P, S], bf16, tag="kT")
            v_sb = persist.tile([P, S], bf16, tag="v")

            # Startup choreography: the first Q matmul needs only x chunks
            # 0-3 and wq chunk-group 0 (~1MB).  Interleave quarter-tile x DMAs
            # with wq chunk-group DMAs across the three rings so the critical
            # first wave isn't queued behind the other ~4MB of inputs.
            xs0 = xpool.tile([P, NCH, TT], bf16, tag="xt", name="xt_0")
            qeng = [nc.sync, nc.gpsimd]
            for cg in range(0, NCH, XSUB):
                qeng[(cg // XSUB) % 2].dma_start(
                    out=xs0[:, cg:cg + XSUB, :], in_=xt_d[0, :, cg:cg + XSUB, :])
                nc.scalar.dma_start(out=wq_sb[:, cg:cg + XSUB, :],
                                    in_=wq[:, cg:cg + XSUB, :])
            nc.scalar.dma_start(out=wk_sb[:], in_=wk[:])
            nc.scalar.dma_start(out=wv_sb[:], in_=wv[:])
            nc.scalar.dma_start(out=identb[:], in_=identb_d[:])
            nc.scalar.dma_start(out=tri_sb[:], in_=tri_d[:])

            def proj_tile(t0):
                ti = t0 // TT
                if ti == 0:
                    xs = xs0
                else:
                    xs = xpool.tile([P, NCH, TT], bf16, tag="xt", name=f"xt_{t0}")
                    nc.sync.dma_start(out=xs[:, :NCH // 2, :],
                                      in_=xt_d[ti, :, :NCH // 2, :])
                    nc.gpsimd.dma_start(out=xs[:, NCH // 2:, :],
                                        in_=xt_d[ti, :, NCH // 2:, :])
                for h in range(HPC):
                    ps = ppsum.tile([P, TT], f32, tag="pp")
                    for c in range(NCH):
                        nc.tensor.matmul(
                            ps[:],
                            lhsT=wq_sb[:, c, h * HD:(h + 1) * HD],
                            rhs=xs[:, c, :],
                            start=(c == 0), stop=(c == NCH - 1),
                        )
                    nc.vector.tensor_copy(qT_sb[:, h, t0:t0 + TT], ps[:])
                ps = ppsum.tile([P, TT], f32, tag="pp")
                for c in range(NCH):
                    nc.tensor.matmul(
                        ps[:], lhsT=wk_sb[:, c, :], rhs=xs[:, c, :],
                        start=(c == 0), stop=(c == NCH - 1),
                    )
                nc.vector.tensor_copy(kT_sb[:, t0:t0 + TT], ps[:])
                ps = ppsum.tile([P, TT], f32, tag="pp")
                for c in range(NCH):
                    nc.tensor.matmul(
                        ps[:], lhsT=wv_sb[:, c, :], rhs=xs[:, c, :],
                        start=(c == 0), stop=(c == NCH - 1),
                    )
                vt = vstage.tile([P, TT], f32r, tag="vt", name=f"vt_{t0}")
                nc.scalar.copy(vt[:], ps[:])
                vtp = popool.tile([P, QT], f32r, tag="po", name=f"vtp_{t0}")
                for j in range(TT // P):
                    nc.tensor.transpose(
                        vtp[:, j * P:(j + 1) * P], vt[:, j * P:(j + 1) * P], identb[:]
                    )
                nc.vector.tensor_copy(v_sb[:, t0:t0 + TT], vtp[:])

            acc_tiles = {}

            def attn_block(q0):
                # Software-pipelined emission: score-groups of head h are
                # interleaved with attn@V chunk-pairs (and DVE sum-adds) of
                # head h-1, so the PE never sits out a full exp latency and
                # the DVE work is spread evenly.
                nkc = n_kchunks(q0)
                ngr = (nkc + GK - 1) // GK
                ets, pos = {}, {}

                def scores_group(h, g0):
                    sp = spsum.tile([P, GK * QT], f32, tag="sp")
                    for kc in range(g0, min(g0 + GK, nkc)):
                        qs = q_start(q0, kc)
                        off = (kc - g0) * QT + (qs - q0)
                        nc.tensor.matmul(
                            sp[:, off:(kc - g0) * QT + QT],
                            lhsT=kT_sb[:, kc * P:(kc + 1) * P],
                            rhs=qT_sb[:, h, qs:q0 + QT],
                            start=True, stop=True,
                        )
                    if variant == "general":
                        mt = mpool.tile([P, GK, QT], bf16, tag="mt")
                        nc.sync.dma_start(
                            out=mt[:],
                            in_=maskT[g0 * P:(g0 + GK) * P, q0:q0 + QT]
                            .rearrange("(g p) q -> p g q", p=P),
                        )
                        nc.vector.tensor_add(out=sp[:], in0=sp[:], in1=mt[:])
                    nc.scalar.activation(
                        out=ets[h][:, g0 * QT:(g0 + GK) * QT], in_=sp[:], func=Exp
                    )
                    if variant == "causal":
                        for kc in range(g0, min(g0 + GK, nkc)):
                            if kc * P >= q0:
                                off = kc * QT + (kc * P - q0)
                                nc.vector.tensor_mul(
                                    out=ets[h][:, off:off + P],
                                    in0=ets[h][:, off:off + P],
                                    in1=tri_sb[:],
                                )

                def attnv_part(h, g0):
                    if h not in pos:
                        pos[h] = popool.tile([P, QT], f32, tag="po",
                                             name=f"po_{q0}_{h}")
                    for kc in range(g0, min(g0 + GK, nkc)):
                        qs = q_start(q0, kc)
                        off = qs - q0
                        nc.tensor.matmul(
                            pos[h][:, off:QT],
                            lhsT=v_sb[:, kc * P:(kc + 1) * P],
                            rhs=ets[h][:, kc * QT + off:kc * QT + QT],
                            start=(kc == 0), stop=(kc == nkc - 1),
                        )

                def finish_head(h):
                    # ot copy rides the scalar engine and is emitted BEFORE
                    # the DVE sum-adds: the po PSUM rotation (which gates the
                    # PE) must never wait behind queued DVE work.
                    accL, accH = acc_tiles[h]
                    ot = ostage.tile([P, QT], bf16, tag="ot", name=f"ot_{q0}_{h}")
                    nc.scalar.copy(ot[:], pos[h][:])
                    nc.gpsimd.dma_start(
                        out=oT_d[h * P:(h + 1) * P, q0:q0 + QT], in_=ot[:]
                    )
                    # softmax partial sums on DVE (bf16, depth <= 8)
                    for kc in range(nkc):
                        qs = q_start(q0, kc)
                        off = qs - q0
                        acc = accL if kc < 8 else accH
                        et_s = ets[h][:, kc * QT + off:kc * QT + QT]
                        dst = acc[:, qs:q0 + QT]
                        if kc == 0 or kc == 8:
                            nc.vector.tensor_copy(dst, et_s)
                        else:
                            nc.vector.tensor_add(out=dst, in0=dst, in1=et_s)
                    # each block's acc column range is final once its adds ran;
                    # stream it out now so the kernel-exit drain isn't stuck
                    # behind 4MB of last-moment acc DMAs
                    nc.gpsimd.dma_start(out=accL_d[h, :, q0:q0 + QT],
                                        in_=accL[:, q0:q0 + QT])
                    if nkc > 8:
                        nc.gpsimd.dma_start(out=accH_d[h, :, q0:q0 + QT],
                                            in_=accH[:, q0:q0 + QT])

                for h in range(HPC):
                    if h not in acc_tiles:
                        acc_tiles[h] = (
                            accpool.tile([P, S], bf16, tag="acc", name=f"accL_{h}"),
                            accpool.tile([P, S], bf16, tag="acc", name=f"accH_{h}"),
                        )
                for h in range(HPC + 1):
                    if h < HPC:
                        ets[h] = etpool.tile([P, NKC * QT], bf16, tag="et",
                                             name=f"et_{q0}_{h}")
                    for g in range(ngr):
                        if h < HPC:
                            scores_group(h, g * GK)
                        if h >= 1:
                            attnv_part(h - 1, g * GK)
                    if h >= 1:
                        finish_head(h - 1)

            # causal pipeline: attention on block t0 needs only tokens <= t0+TT
            for t0 in range(0, S, TT):
                proj_tile(t0)
                attn_block(t0)

    nc.compile()
    return nc


def get_nc(variant="causal"):
    if variant not in _CACHE:
        _CACHE[variant] = _build_nc(variant)
    return _CACHE[variant]


def detect_variant(attention_mask):
    m = np.asarray(attention_mask, dtype=np.float32)[:, 0]   # [B, S, S] (q, k)
    if not np.any(m):
        return "zeros"
    kk = np.arange(S)
    lower = kk[None, :] <= kk[:, None]                       # [S(q), S(k)]
    for b in range(m.shape[0]):
        if np.any(m[b][lower] != 0.0):
            return "general"
        if np.any(m[b][~lower] > -1e8):
            return "general"
    return "causal"


def make_in_maps(hidden_states, attention_mask, Wq, Wk, Wv, variant=None):
    import ml_dtypes

    if variant is None:
        variant = detect_variant(attention_mask)
    bf = ml_dtypes.bfloat16
    x = np.asarray(hidden_states, dtype=np.float32)
    # [b] -> [S//TT, P, NCH, TT]: x[b].T is [HID, S]; hid -> (c p), s -> (ti t)
    xTb = [np.ascontiguousarray(
        x[b].T.reshape(NCH, P, S // TT, TT).transpose(2, 1, 0, 3)
    ).astype(bf) for b in range(B)]
    wq_s = (np.asarray(Wq, dtype=np.float32) / math.sqrt(HD)).astype(bf)
    wk = np.asarray(Wk, dtype=np.float32).astype(bf)
    wv = np.asarray(Wv, dtype=np.float32).astype(bf)

    def warr(w):  # [HID, F] -> [P, NCH, F]
        return np.ascontiguousarray(w.reshape(NCH, P, -1).transpose(1, 0, 2))
    identb = np.eye(P, dtype=np.float32)
    # tri[p, j] = 1 if p <= j else 0  (keep k <= q within the diagonal chunk)
    tri = np.triu(np.ones((P, P), dtype=np.float32)).astype(bf)
    if variant == "general":
        mT = [np.ascontiguousarray(
            np.asarray(attention_mask, dtype=np.float32)[b, 0].T).astype(bf)
            for b in range(B)]

    in_maps = []
    for c in range(NCORES):
        b, kv = c % 2, c // 2
        m = {
            "xt": xTb[b],
            "wq": warr(wq_s[:, kv * FPC:(kv + 1) * FPC]),
            "wk": warr(wk[:, kv * KVW:(kv + 1) * KVW]),
            "wv": warr(wv[:, kv * KVW:(kv + 1) * KVW]),
            "identb": identb,
            "tri": tri,
        }
        if variant == "general":
            m["maskT"] = mT[b]
        in_maps.append(m)
    return in_maps


def postprocess(res, variant):
    """Assemble full [B, S, HID] f32 output from per-core oT/accL/accH."""
    out = np.empty((B, S, HID), dtype=np.float32)
    for c in range(NCORES):
        b, kv = c % 2, c // 2
        oT = res.results[c]["oT"].astype(np.float64)          # [FPC, S]
        aL = res.results[c]["accL"].astype(np.float64)        # [HPC, P, S]
        aH = res.results[c]["accH"].astype(np.float64)
        if variant == "causal":
            # k-chunks 8..15 only reach q >= 1024; cols below hold garbage
            aH = aH.copy()
            aH[..., :8 * P] = 0.0
        sums = aL.sum(axis=1) + aH.sum(axis=1)                # [HPC, S]
        o = (oT.reshape(HPC, HD, S) / sums[:, None, :])       # [HPC, HD, S]
        out[b, :, kv * FPC:(kv + 1) * FPC] = (
            o.transpose(2, 0, 1).reshape(S, FPC).astype(np.float32)
        )
    return out


def run_on_cores(inputs, trace=False, tmpdir=None):
    from concourse.bass_utils import run_bass_kernel_spmd

    variant = detect_variant(inputs["attention_mask"])
    nc = get_nc(variant)
    in_maps = make_in_maps(**inputs, variant=variant)
    kw = {}
    if trace:
        kw = {"trace": True, "tmpdir": tmpdir}
    res = run_bass_kernel_spmd(nc, in_maps, core_ids=list(range(NCORES)), **kw)
    return postprocess(res, variant), res


def kernel(hidden_states, attention_mask, Wq, Wk, Wv):
    out, _ = run_on_cores({
        "hidden_states": hidden_states,
        "attention_mask": attention_mask,
        "Wq": Wq, "Wk": Wk, "Wv": Wv,
    })
    return out
